# revision 2
# baseline (speedup 1.0000x reference)
"""Causal multi-head attention block (QKV proj -> causal MHA -> out proj) on 8 Trainium2
cores — v3.

Sharding: core = b*2 + hh handles batch b (of 4) and head-half hh (8 of 16 heads).
Host pre-transposes x (x^T) and pre-formats all weights as bf16, so the device does
only GEMMs + attention. All matmuls run in bf16 (fp32 PSUM accumulation); softmax is
fp32 on the Activation engine.

Structure per core:
  - Q^T/K^T [feat(part), tok] and V [tok(part), feat] projections from resident x^T.
  - scores^T tiles [k(part), q] with the causal mask added via an extra accumulate
    matmul (identity x neg-triangle) on the PE, then exp on Act into bf16 P tiles.
  - AV with P stationary: Y[q(part), 65] accumulated over k-blocks; col 64 is the
    softmax denominator via a ones-column in V. Normalize = DVE reciprocal +
    per-partition tensor_scalar multiply; y^T via PE transposes; out-proj into
    token-major z with a pairwise ReduceScatter.
  - The Act engine runs only exp and is the secondary bottleneck; attention units
    are interleaved with fine-grained QKV / out-proj filler via a deficit model so
    the PE never waits on exp.
"""

import numpy as np

import concourse.bass as bass
import concourse.tile as tile
from concourse import bacc, mybir
from concourse.bass_utils import run_bass_kernel_spmd

F32 = mybir.dt.float32
F32R = mybir.dt.float32r
BF16 = mybir.dt.bfloat16
AF = mybir.ActivationFunctionType

B, T, C, H = 4, 2048, 1024, 16
D = C // H          # 64
NHL = H // 2        # 8 local heads per core
NHP = NHL // 2      # 4 local head pairs
FL = NHL * D        # 512 local features
NCC = C // 128      # 8 contraction chunks over C
NTB = T // 128      # 16 token blocks
NTT = T // 512      # 4 token tiles / qtiles
VE = D + 1          # 65: V feature cols + ones column
NEG = -1.0e30

# zpart row remap so each pairwise-RS chunk is a contiguous 256-row block:
# chunk c holds tb c (rank0 tokens) then tb 8+c (rank1 tokens)
ZROW = {}
for _c in range(8):
    ZROW[_c] = _c * 256
    ZROW[8 + _c] = _c * 256 + 128


def build():
    nc = bacc.Bacc("TRN2", target_bir_lowering=False, num_devices=8)

    xt_d = nc.dram_tensor("xt", [C, T], BF16, kind="ExternalInput")
    wq_d = nc.dram_tensor("wq", [NCC, 128, FL], BF16, kind="ExternalInput")
    wk_d = nc.dram_tensor("wk", [NCC, 128, FL], BF16, kind="ExternalInput")
    wv_d = nc.dram_tensor("wv", [NCC, 128, FL], BF16, kind="ExternalInput")
    wo_d = nc.dram_tensor("wo", [NHP, 128, C], BF16, kind="ExternalInput")
    bq_d = nc.dram_tensor("bq", [128, NHP], F32, kind="ExternalInput")
    bk_d = nc.dram_tensor("bk", [128, NHP], F32, kind="ExternalInput")
    id_d = nc.dram_tensor("ident", [128, 128], BF16, kind="ExternalInput")
    neg_d = nc.dram_tensor("negtri", [128, 128], BF16, kind="ExternalInput")
    idr_d = nc.dram_tensor("identr", [128, 128], F32R, kind="ExternalInput")
    ones_d = nc.dram_tensor("vones", [128, NTB * NHL], BF16, kind="ExternalInput")
    zh = nc.dram_tensor("zh", [T // 2, C], F32, kind="ExternalOutput")

    with tile.TileContext(nc) as tc:
        with (
            tc.tile_pool(name="res", bufs=1) as res,
            tc.tile_pool(name="cst", bufs=1) as cst,
            tc.tile_pool(name="dram", bufs=1, space="DRAM") as dram,
            tc.tile_pool(name="mp", bufs=2, space="PSUM") as mp,       # 2 banks
            tc.tile_pool(name="sp", bufs=2, space="PSUM") as sp,       # 4 banks
            tc.tile_pool(name="yp", bufs=2, space="PSUM") as yp,       # 2 banks
            tc.tile_pool(name="pp", bufs=8) as pp,
            tc.tile_pool(name="yb", bufs=6) as yb,
            tc.tile_pool(name="rc", bufs=6) as rc,
            tc.tile_pool(name="zb", bufs=3) as zb,
        ):
            # ---------------- resident tiles ----------------
            xt = res.tile([128, NCC * T], BF16)          # x^T  [c | cc, tok]
            xt3 = xt[:].rearrange("p (c t) -> p c t", c=NCC)
            qt_sb = res.tile([128, NHP * T], BF16)       # Q^T  [d(2 heads) | hp, tok]
            qt3 = qt_sb[:].rearrange("p (h t) -> p h t", h=NHP)
            kt_sb = res.tile([128, NHP * T], BF16)       # K^T
            kt3 = kt_sb[:].rearrange("p (h t) -> p h t", h=NHP)
            v_sb = res.tile([128, NTB * NHL * VE], BF16)  # V' [k | tb, h, d+1]
            v4 = v_sb[:].rearrange("p (b h e) -> p b h e", b=NTB, h=NHL)
            yt_sb = res.tile([128, NHP * T], BF16)       # y^T [f(2 heads) | hp, tok]
            yt3 = yt_sb[:].rearrange("p (h t) -> p h t", h=NHP)

            zpart = dram.tile([T, C], F32, name="zpart")
            zreds = [dram.tile([128, C], F32, name=f"zred{i}") for i in range(8)]

            # ------------- loads, ordered so K(tt0)/V(tt0) unblock fastest -------------
            scr = cst.tile([128, 128], BF16, tag="scr")
            nc.vector.memset(scr[:], 0.0)
            nc.sync.dma_start(
                xt3[:, :, 0:512],
                xt_d[:, 0:512].rearrange("(c p) t -> p c t", p=128),
            )
            wk_sb = cst.tile([128, NCC * FL], BF16, tag="wk")
            wk3 = wk_sb[:].rearrange("p (c f) -> p c f", c=NCC)
            nc.sync.dma_start(
                wk3[:, :, 0:256], wk_d[:, :, 0:256].rearrange("c p f -> p c f")
            )
            bk_sb = cst.tile([128, NHP], F32, tag="bk")
            nc.sync.dma_start(bk_sb[:], bk_d[:, :])
            nc.sync.dma_start(
                wk3[:, :, 256:512], wk_d[:, :, 256:512].rearrange("c p f -> p c f")
            )
            id_sb = cst.tile([128, 128], BF16, tag="id")
            nc.sync.dma_start(id_sb[:], id_d[:, :])
            neg_sb = cst.tile([128, 128], BF16, tag="neg")
            nc.sync.dma_start(neg_sb[:], neg_d[:, :])
            idr_sb = cst.tile([128, 128], F32R, tag="idr")
            nc.sync.dma_start(idr_sb[:], idr_d[:, :])
            bq_sb = cst.tile([128, NHP], F32, tag="bq")
            nc.sync.dma_start(bq_sb[:], bq_d[:, :])
            wv_sb = cst.tile([128, NCC * FL], BF16, tag="wv")
            wv3 = wv_sb[:].rearrange("p (c f) -> p c f", c=NCC)
            nc.sync.dma_start(wv3, wv_d[:, :, :].rearrange("c p f -> p c f"))
            wq_sb = cst.tile([128, NCC * FL], BF16, tag="wq")
            wq3 = wq_sb[:].rearrange("p (c f) -> p c f", c=NCC)
            nc.sync.dma_start(wq3, wq_d[:, :, :].rearrange("c p f -> p c f"))
            # warm the exp table set (hides ACT_TABLE_LOAD)
            warm = cst.tile([1, 1], F32, tag="warm")
            nc.scalar.activation(warm[:], bq_sb[0:1, 0:1], AF.Exp)
            # ones column of V' via a small DMA + one strided DVE copy
            ones_sb = cst.tile([128, NTB * NHL], BF16, tag="ones")
            nc.sync.dma_start(ones_sb[:], ones_d[:, :])
            nc.vector.tensor_copy(
                v4[:, :, :, D:D + 1],
                ones_sb[:].rearrange("p (b h) -> p b h", b=NTB).unsqueeze(3),
            )
            # PE p-state warm-up while the x^T/weight DMAs land (scratch is
            # memset-initialized, so no DMA gates the first matmul)
            wps = sp.tile([128, 1024], F32, tag="s", name="warmps")
            for wi in range(100):
                nc.tensor.matmul(
                    wps[:, (wi % 8) * 128:(wi % 8) * 128 + 128],
                    scr[:], scr[:], start=True, stop=True,
                )
            for tt in range(1, NTT):
                nc.sync.dma_start(
                    xt3[:, :, tt * 512:(tt + 1) * 512],
                    xt_d[:, tt * 512:(tt + 1) * 512].rearrange(
                        "(c p) t -> p c t", p=128
                    ),
                )
            wo_sb = cst.tile([128, NHP * C], BF16, tag="wo")
            wo3 = wo_sb[:].rearrange("p (h f) -> p h f", h=NHP)
            nc.sync.dma_start(wo3, wo_d[:, :, :].rearrange("h p f -> p h f"))

            # ---------------- fine-grained emission units ----------------
            # Each filler unit is (pe_ns_estimate, closure). The scheduler pops
            # fillers to cover the Act-vs-PE deficit inside attention units.
            def u_qk(tt, fb, w3, b_sb, dst3, nm):
                key = (nm, tt, fb)
                def go():
                    ps = mp.tile([128, 512], F32, tag="mp", name=f"{nm}{tt}_{fb}")
                    for cc in range(NCC):
                        nc.tensor.matmul(
                            ps[:],
                            w3[:, cc, fb * 128:(fb + 1) * 128],
                            xt3[:, cc, tt * 512:(tt + 1) * 512],
                            start=(cc == 0),
                            stop=(cc == NCC - 1),
                        )
                    nc.vector.tensor_scalar_add(
                        dst3[:, fb, tt * 512:(tt + 1) * 512], ps[:],
                        b_sb[:, fb:fb + 1],
                    )
                return (key, 1707, go)

            def u_v(tb):
                key = ("vt", tb)
                def go():
                    ps = mp.tile([128, 512], F32, tag="mp", name=f"v{tb}")
                    for cc in range(NCC):
                        nc.tensor.matmul(
                            ps[:],
                            xt3[:, cc, tb * 128:(tb + 1) * 128],
                            wv3[:, cc, :],
                            start=(cc == 0),
                            stop=(cc == NCC - 1),
                        )
                    nc.vector.tensor_copy(
                        v4[:, tb, :, 0:D],
                        ps[:].rearrange("p (h d) -> p h d", h=NHL),
                    )
                return (key, 1707, go)

            def u_op(tb, ct, on_act=False):
                key = ("op", tb, ct)
                def go():
                    zps = mp.tile([128, 512], F32, tag="mp", name=f"z{tb}_{ct}")
                    for cc in range(NHP):
                        nc.tensor.matmul(
                            zps[:],
                            yt3[:, cc, tb * 128:(tb + 1) * 128],
                            wo3[:, cc, ct * 512:(ct + 1) * 512],
                            start=(cc == 0),
                            stop=(cc == NHP - 1),
                        )
                    z_sb = zb.tile([128, 512], F32, tag="z", name=f"zs{tb}_{ct}")
                    if on_act:
                        nc.scalar.copy(z_sb[:], zps[:])
                    else:
                        nc.vector.tensor_copy(z_sb[:], zps[:])
                    nc.sync.dma_start(
                        zpart[ZROW[tb]:ZROW[tb] + 128, ct * 512:(ct + 1) * 512],
                        z_sb[:],
                    )
                return (key, 880, go)

            fillers = []          # deque of (key, pe_ns, closure)
            done_keys = set()
            deficit = [0.0]       # act-over-pe backlog inside attention

            def fill(need_ns):
                deficit[0] += need_ns
                while fillers and deficit[0] > fillers[0][1] * 2.0:
                    key, pe_ns, go = fillers.pop(0)
                    go()
                    done_keys.add(key)
                    deficit[0] -= pe_ns

            def require(*keys):
                # run exactly the queued fillers the caller depends on
                need = {k for k in keys if k not in done_keys}
                if not need:
                    return
                rest = []
                forced = []
                for key, pe_ns, go in fillers:
                    if key in need:
                        go()
                        done_keys.add(key)
                        deficit[0] -= pe_ns
                        need.discard(key)
                        forced.append(key)
                    else:
                        rest.append((key, pe_ns, go))
                fillers[:] = rest

                if need:
                    raise RuntimeError(f"missing filler deps: {need}")

            pend_tp = []          # pending y-transpose closures (pair-complete)

            def flush_tp(keep=0):
                while len(pend_tp) > keep:
                    pend_tp.pop(0)()

            ysave = {}

            def emit_att(h, qt):
                hp, hi = h // 2, h % 2
                n_kb = 4 * (qt + 1)
                require(("q", qt, hp),
                        *[("k", t, hp) for t in range(qt + 1)],
                        *[("vt", tb) for tb in range(4 * qt + 4)])
                flush_tp(keep=1)
                # one psum bank; a single accumulation group brackets all four
                # qb sub-regions (start marks the bank pending-zero, first
                # write per byte overwrites, later writes accumulate)
                Y = yp.tile([128, 512], F32, tag="y", name=f"Y{h}_{qt}")
                Y3 = Y[:, 0:4 * VE].rearrange("p (q e) -> p q e", q=4)
                for kbp in range(n_kb // 2):
                    s = sp.tile([128, 1024], F32, tag="s", name=f"s{h}_{qt}_{kbp}")
                    cs = []
                    pe_ns = 0.0
                    for c2 in range(2):
                        kb = 2 * kbp + c2
                        c = kb - 4 * qt
                        cs.append(c)
                        j0 = c * 128 if c > 0 else 0
                        nc.tensor.matmul(
                            s[:, c2 * 512 + j0:(c2 + 1) * 512],
                            kt3[hi * 64:(hi + 1) * 64, hp, kb * 128:(kb + 1) * 128],
                            qt3[hi * 64:(hi + 1) * 64, hp, qt * 512 + j0:(qt + 1) * 512],
                            tile_position=(hi * 64, 0),
                            start=True,
                            stop=not (0 <= c <= 3),
                        )
                        pe_ns += (512 - j0) * 0.4167
                        if 0 <= c <= 3:
                            nc.tensor.matmul(
                                s[:, c2 * 512 + c * 128: c2 * 512 + (c + 1) * 128],
                                id_sb[:],
                                neg_sb[:],
                                start=False,
                                stop=True,
                            )
                            pe_ns += 128 * 0.4167
                    P = pp.tile([128, 1024], BF16, tag="p", name=f"p{h}_{qt}_{kbp}")
                    if 0 <= cs[0] <= 3:
                        # diagonal pair: exp the two written ranges, skipping
                        # the never-written fully-masked hole between blocks
                        ja = cs[0] * 128
                        jb = 512 + cs[1] * 128
                        nc.scalar.activation(
                            P[:, ja:512], s[:, ja:512], AF.Exp, scale=0.125
                        )
                        nc.scalar.activation(
                            P[:, jb:], s[:, jb:], AF.Exp, scale=0.125
                        )
                        act_ns = (1536 - ja - jb) * 0.833 + 500
                    else:
                        nc.scalar.activation(P[:], s[:], AF.Exp, scale=0.125)
                        act_ns = 1024 * 0.833 + 250
                    for c2 in range(2):
                        kb = 2 * kbp + c2
                        c = kb - 4 * qt
                        for qb in range(4):
                            if c > qb:
                                continue
                            nc.tensor.matmul(
                                Y3[:, qb, :],
                                P[:, c2 * 512 + qb * 128: c2 * 512 + (qb + 1) * 128],
                                v4[:, kb, h, :],
                                start=(kb == 0 and qb == 0),
                                stop=(kb == n_kb - 1 and qb == 3),
                            )
                            pe_ns += VE * 0.4167
                    fill(max(0.0, act_ns - pe_ns))
                # normalize: y = Y[:, :64] / Y[:, 64]
                r = rc.tile([128, 4], F32, tag="r", name=f"r{h}_{qt}")
                nc.vector.reciprocal(
                    r[:], Y3[:, :, D:D + 1].rearrange("p q e -> p (q e)")
                )
                y = yb.tile([128, 4 * D], BF16, tag="yb", name=f"y{h}_{qt}")
                y3 = y[:].rearrange("p (q d) -> p q d", q=4)
                for qb in range(4):
                    nc.vector.tensor_scalar(
                        y3[:, qb, :], Y3[:, qb, 0:D], r[:, qb:qb + 1], None,
                        mybir.AluOpType.mult,
                    )
                if hi == 0:
                    ysave[(h, qt)] = y
                else:
                    ye = ysave.pop((h - 1, qt))

                    def tp_go(ye=ye, y=y, hp=hp, qt=qt):
                        ye3 = ye[:].rearrange("p (q d) -> p q d", q=4)
                        yo3 = y[:].rearrange("p (q d) -> p q d", q=4)
                        tp = sp.tile([128, 2048], BF16, tag="s", name=f"yt{hp}_{qt}")
                        for qb in range(4):
                            nc.tensor.transpose(
                                tp[0:64, qb * 128:(qb + 1) * 128], ye3[:, qb, :],
                                id_sb[:],
                            )
                            nc.tensor.transpose(
                                tp[64:128, qb * 128:(qb + 1) * 128], yo3[:, qb, :],
                                id_sb[:],
                            )
                        nc.vector.tensor_copy(
                            yt3[:, hp, qt * 512:(qt + 1) * 512], tp[:, 0:512]
                        )
                    pend_tp.append(tp_go)

            def emit_rs(c):
                nc.gpsimd.collective_compute(
                    "ReduceScatter",
                    mybir.AluOpType.add,
                    replica_groups=[[0, 1], [2, 3], [4, 5], [6, 7]],
                    ins=[zpart[c * 256:(c + 1) * 256, :].opt()],
                    outs=[zreds[c].opt()],
                )
                nc.sync.dma_start(zh[c * 128:(c + 1) * 128, :], zreds[c][:])

            # ---------------- static schedule ----------------
            def push(*units):
                fillers.extend(units)

            def run_now(*units):
                for key, _, go in units:
                    go()
                    done_keys.add(key)

            # K/V/Q for tt0 emitted directly (attention qt0 needs them)
            run_now(*[u_qk(0, fb, wk3, bk_sb, kt3, "k") for fb in range(NHP)])
            run_now(*[u_v(tb) for tb in range(4)])
            run_now(*[u_qk(0, fb, wq3, bq_sb, qt3, "q") for fb in range(NHP)])

            # remaining QKV work becomes filler, in dependency-safe order
            push(*[u_qk(1, fb, wk3, bk_sb, kt3, "k") for fb in range(NHP)])
            push(*[u_v(tb) for tb in range(4, 8)])
            push(*[u_qk(1, fb, wq3, bq_sb, qt3, "q") for fb in range(NHP)])
            push(*[u_qk(2, fb, wk3, bk_sb, kt3, "k") for fb in range(NHP)])
            push(*[u_v(tb) for tb in range(8, 12)])
            push(*[u_qk(2, fb, wq3, bq_sb, qt3, "q") for fb in range(NHP)])

            for h in range(NHL):
                emit_att(h, 0)
            # qt0 done -> out-proj tb0-3 available; also prepare tt3 inputs
            push(*[u_qk(3, fb, wk3, bk_sb, kt3, "k") for fb in range(NHP)])
            push(*[u_v(tb) for tb in range(12, 16)])
            push(*[u_qk(3, fb, wq3, bq_sb, qt3, "q") for fb in range(NHP)])
            flush_tp()
            push(*[u_op(tb, ct) for tb in range(0, 4) for ct in range(2)])

            for h in range(NHL):
                emit_att(h, 1)
            flush_tp()
            push(*[u_op(tb, ct) for tb in range(4, 8) for ct in range(2)])

            # interleave qt2 and qt3 heads; qt3 needs K/V/Q(tt3) which sit at
            # the front of the filler queue by now
            lead = [(0, 2), (1, 2)]
            tail_o = [(6, 3), (7, 3)]
            mid = []
            for i in range(NHL - 2):
                mid.append((i, 3))
                mid.append((i + 2, 2))
            order = lead + mid + tail_o
            for h, qt in order:
                emit_att(h, qt)
            flush_tp()
            push(*[u_op(tb, ct, on_act=(ct == 1)) for tb in range(8, 12) for ct in range(2)])

            # drain remaining fillers (anything the deficit model didn't pull)
            while fillers:
                fillers.pop(0)[2]()
            for c in range(4):
                emit_rs(c)
            for tb in range(12, 16):
                # drain-region out-proj: both halves in one wide s-pool psum
                # tile, copies split across DVE and Act so they overlap
                zps = sp.tile([128, 1024], F32, tag="s", name=f"zw{tb}")
                for ct in range(2):
                    for cc in range(NHP):
                        nc.tensor.matmul(
                            zps[:, ct * 512:(ct + 1) * 512],
                            yt3[:, cc, tb * 128:(tb + 1) * 128],
                            wo3[:, cc, ct * 512:(ct + 1) * 512],
                            start=(cc == 0),
                            stop=(cc == NHP - 1),
                        )
                z_sb = zb.tile([128, 1024], F32, tag="zw", name=f"zsw{tb}")
                nc.vector.tensor_copy(z_sb[:, 0:512], zps[:, 0:512])
                nc.sync.dma_start(
                    zpart[ZROW[tb]:ZROW[tb] + 128, 0:512], z_sb[:, 0:512]
                )
                nc.scalar.copy(z_sb[:, 512:1024], zps[:, 512:1024])
                nc.sync.dma_start(
                    zpart[ZROW[tb]:ZROW[tb] + 128, 512:1024], z_sb[:, 512:1024]
                )
                emit_rs(tb - 8)

    nc.compile()
    return nc


_NC_CACHE = None


def _get_nc():
    global _NC_CACHE
    if _NC_CACHE is None:
        _NC_CACHE = build()
    return _NC_CACHE


def _bf16(a):
    import ml_dtypes
    return np.ascontiguousarray(a).astype(ml_dtypes.bfloat16)


def _in_maps(x, Wqkv, bqkv, Wo, bo):
    x = np.asarray(x, dtype=np.float32)
    Wqkv = np.asarray(Wqkv, dtype=np.float32)
    bqkv = np.asarray(bqkv, dtype=np.float32)
    Wo = np.asarray(Wo, dtype=np.float32)
    bo = np.asarray(bo, dtype=np.float32)

    ident = np.eye(128, dtype=np.float32)
    i_ = np.arange(128)[:, None]
    j_ = np.arange(128)[None, :]
    negtri = np.where(i_ > j_, np.float32(NEG), np.float32(0.0))

    in_maps = []
    for core in range(8):
        b, hh = core // 2, core % 2
        sl = slice(hh * FL, (hh + 1) * FL)
        wq = Wqkv[:, 0 * C:1 * C][:, sl]
        wk = Wqkv[:, 1 * C:2 * C][:, sl]
        wv = Wqkv[:, 2 * C:3 * C][:, sl]
        wo = Wo[sl, :]
        in_maps.append({
            "xt": _bf16(x[b].T),
            "wq": _bf16(wq.reshape(NCC, 128, FL)),
            "wk": _bf16(wk.reshape(NCC, 128, FL)),
            "wv": _bf16(wv.reshape(NCC, 128, FL)),
            "wo": _bf16(wo.reshape(NHP, 128, C)),
            "bq": np.ascontiguousarray(bqkv[0 * C:1 * C][sl].reshape(NHP, 128).T),
            "bk": np.ascontiguousarray(bqkv[1 * C:2 * C][sl].reshape(NHP, 128).T),
            "ident": _bf16(ident),
            "negtri": _bf16(negtri),
            "identr": ident,
            "vones": _bf16(np.ones((128, NTB * NHL), dtype=np.float32)),
        })
    return in_maps


def _assemble(res, bias):
    out = np.empty((B, T, C), dtype=np.float32)
    for b in range(B):
        out[b, : T // 2] = res.results[2 * b]["zh"]
        out[b, T // 2:] = res.results[2 * b + 1]["zh"]
    out += bias[None, None, :]
    return out


def kernel(x, Wqkv, bqkv, Wo, bo):
    in_maps = _in_maps(x, Wqkv, bqkv, Wo, bo)
    res = run_bass_kernel_spmd(_get_nc(), in_maps, core_ids=list(range(8)))
    # v-bias and output bias fold to a constant vector: softmax weights sum to
    # 1, so y = P@(v+bv)/rs = P@v/rs + bv  ->  out += bo + bv @ Wo
    bqkv_ = np.asarray(bqkv, dtype=np.float32)
    bias = (np.asarray(bo, dtype=np.float32)
            + bqkv_[2 * C:3 * C] @ np.asarray(Wo, dtype=np.float32))
    return _assemble(res, bias)


def run_traced(x, Wqkv, bqkv, Wo, bo, trace_cores=None):
    in_maps = _in_maps(x, Wqkv, bqkv, Wo, bo)
    res = run_bass_kernel_spmd(
        _get_nc(), in_maps, core_ids=list(range(8)), trace=True,
        trace_cores=trace_cores,
    )
    return res


# revision 3
# speedup vs baseline: 1.0063x; 1.0063x over previous
"""Causal multi-head attention block (QKV proj -> causal MHA -> out proj) on 8 Trainium2
cores — v3.

Sharding: core = b*2 + hh handles batch b (of 4) and head-half hh (8 of 16 heads).
Host pre-transposes x (x^T) and pre-formats all weights as bf16, so the device does
only GEMMs + attention. All matmuls run in bf16 (fp32 PSUM accumulation); softmax is
fp32 on the Activation engine.

Structure per core:
  - Q^T/K^T [feat(part), tok] and V [tok(part), feat] projections from resident x^T.
  - scores^T tiles [k(part), q] with the causal mask added via an extra accumulate
    matmul (identity x neg-triangle) on the PE, then exp on Act into bf16 P tiles.
  - AV with P stationary: Y[q(part), 65] accumulated over k-blocks; col 64 is the
    softmax denominator via a ones-column in V. Normalize = DVE reciprocal +
    per-partition tensor_scalar multiply; y^T via PE transposes; out-proj into
    token-major z with a pairwise ReduceScatter.
  - The Act engine runs only exp and is the secondary bottleneck; attention units
    are interleaved with fine-grained QKV / out-proj filler via a deficit model so
    the PE never waits on exp.
"""

import numpy as np

import concourse.bass as bass
import concourse.tile as tile
from concourse import bacc, mybir
from concourse.bass_utils import run_bass_kernel_spmd

F32 = mybir.dt.float32
F32R = mybir.dt.float32r
BF16 = mybir.dt.bfloat16
AF = mybir.ActivationFunctionType

B, T, C, H = 4, 2048, 1024, 16
D = C // H          # 64
NHL = H // 2        # 8 local heads per core
NHP = NHL // 2      # 4 local head pairs
FL = NHL * D        # 512 local features
NCC = C // 128      # 8 contraction chunks over C
NTB = T // 128      # 16 token blocks
NTT = T // 512      # 4 token tiles / qtiles
VE = D + 1          # 65: V feature cols + ones column
NEG = -1.0e30

# zpart row remap so each pairwise-RS chunk is a contiguous 256-row block:
# chunk c holds tb c (rank0 tokens) then tb 8+c (rank1 tokens)
ZROW = {}
for _c in range(8):
    ZROW[_c] = _c * 256
    ZROW[8 + _c] = _c * 256 + 128


def build():
    nc = bacc.Bacc("TRN2", target_bir_lowering=False, num_devices=8)

    xt_d = nc.dram_tensor("xt", [C, T], BF16, kind="ExternalInput")
    wq_d = nc.dram_tensor("wq", [NCC, 128, FL], BF16, kind="ExternalInput")
    wk_d = nc.dram_tensor("wk", [NCC, 128, FL], BF16, kind="ExternalInput")
    wv_d = nc.dram_tensor("wv", [NCC, 128, FL], BF16, kind="ExternalInput")
    wo_d = nc.dram_tensor("wo", [NHP, 128, C], BF16, kind="ExternalInput")
    bq_d = nc.dram_tensor("bq", [128, NHP], F32, kind="ExternalInput")
    bk_d = nc.dram_tensor("bk", [128, NHP], F32, kind="ExternalInput")
    id_d = nc.dram_tensor("ident", [128, 128], BF16, kind="ExternalInput")
    neg_d = nc.dram_tensor("negtri", [128, 128], BF16, kind="ExternalInput")
    idr_d = nc.dram_tensor("identr", [128, 128], F32R, kind="ExternalInput")
    ones_d = nc.dram_tensor("vones", [128, NTB * NHL], BF16, kind="ExternalInput")
    zh = nc.dram_tensor("zh", [T // 2, C], F32, kind="ExternalOutput")

    with tile.TileContext(nc) as tc:
        with (
            tc.tile_pool(name="res", bufs=1) as res,
            tc.tile_pool(name="cst", bufs=1) as cst,
            tc.tile_pool(name="dram", bufs=1, space="DRAM") as dram,
            tc.tile_pool(name="mp", bufs=2, space="PSUM") as mp,       # 2 banks
            tc.tile_pool(name="sp", bufs=2, space="PSUM") as sp,       # 4 banks
            tc.tile_pool(name="yp", bufs=2, space="PSUM") as yp,       # 2 banks
            tc.tile_pool(name="pp", bufs=8) as pp,
            tc.tile_pool(name="yb", bufs=6) as yb,
            tc.tile_pool(name="rc", bufs=6) as rc,
            tc.tile_pool(name="zb", bufs=3) as zb,
        ):
            # ---------------- resident tiles ----------------
            xt = res.tile([128, NCC * T], BF16)          # x^T  [c | cc, tok]
            xt3 = xt[:].rearrange("p (c t) -> p c t", c=NCC)
            qt_sb = res.tile([128, NHP * T], BF16)       # Q^T  [d(2 heads) | hp, tok]
            qt3 = qt_sb[:].rearrange("p (h t) -> p h t", h=NHP)
            kt_sb = res.tile([128, NHP * T], BF16)       # K^T
            kt3 = kt_sb[:].rearrange("p (h t) -> p h t", h=NHP)
            v_sb = res.tile([128, NTB * NHL * VE], BF16)  # V' [k | tb, h, d+1]
            v4 = v_sb[:].rearrange("p (b h e) -> p b h e", b=NTB, h=NHL)
            yt_sb = res.tile([128, NHP * T], BF16)       # y^T [f(2 heads) | hp, tok]
            yt3 = yt_sb[:].rearrange("p (h t) -> p h t", h=NHP)

            zpart = dram.tile([T, C], F32, name="zpart")
            zreds = [dram.tile([128, C], F32, name=f"zred{i}") for i in range(8)]

            # ------------- loads, ordered so K(tt0)/V(tt0) unblock fastest -------------
            scr = cst.tile([128, 128], BF16, tag="scr")
            nc.vector.memset(scr[:], 0.0)
            nc.sync.dma_start(
                xt3[:, :, 0:512],
                xt_d[:, 0:512].rearrange("(c p) t -> p c t", p=128),
            )
            wk_sb = cst.tile([128, NCC * FL], BF16, tag="wk")
            wk3 = wk_sb[:].rearrange("p (c f) -> p c f", c=NCC)
            nc.sync.dma_start(
                wk3[:, :, 0:256], wk_d[:, :, 0:256].rearrange("c p f -> p c f")
            )
            bk_sb = cst.tile([128, NHP], F32, tag="bk")
            nc.sync.dma_start(bk_sb[:], bk_d[:, :])
            nc.sync.dma_start(
                wk3[:, :, 256:512], wk_d[:, :, 256:512].rearrange("c p f -> p c f")
            )
            id_sb = cst.tile([128, 128], BF16, tag="id")
            nc.sync.dma_start(id_sb[:], id_d[:, :])
            neg_sb = cst.tile([128, 128], BF16, tag="neg")
            nc.sync.dma_start(neg_sb[:], neg_d[:, :])
            idr_sb = cst.tile([128, 128], F32R, tag="idr")
            nc.sync.dma_start(idr_sb[:], idr_d[:, :])
            bq_sb = cst.tile([128, NHP], F32, tag="bq")
            nc.sync.dma_start(bq_sb[:], bq_d[:, :])
            wv_sb = cst.tile([128, NCC * FL], BF16, tag="wv")
            wv3 = wv_sb[:].rearrange("p (c f) -> p c f", c=NCC)
            nc.sync.dma_start(wv3, wv_d[:, :, :].rearrange("c p f -> p c f"))
            wq_sb = cst.tile([128, NCC * FL], BF16, tag="wq")
            wq3 = wq_sb[:].rearrange("p (c f) -> p c f", c=NCC)
            nc.sync.dma_start(wq3, wq_d[:, :, :].rearrange("c p f -> p c f"))
            # warm the exp table set (hides ACT_TABLE_LOAD)
            warm = cst.tile([1, 1], F32, tag="warm")
            nc.scalar.activation(warm[:], bq_sb[0:1, 0:1], AF.Exp)
            # ones column of V' via a small DMA + one strided DVE copy
            ones_sb = cst.tile([128, NTB * NHL], BF16, tag="ones")
            nc.sync.dma_start(ones_sb[:], ones_d[:, :])
            nc.vector.tensor_copy(
                v4[:, :, :, D:D + 1],
                ones_sb[:].rearrange("p (b h) -> p b h", b=NTB).unsqueeze(3),
            )
            # PE p-state warm-up while the x^T/weight DMAs land (scratch is
            # memset-initialized, so no DMA gates the first matmul)
            wps = sp.tile([128, 1024], F32, tag="s", name="warmps")
            for wi in range(80):
                nc.tensor.matmul(
                    wps[:, (wi % 8) * 128:(wi % 8) * 128 + 128],
                    scr[:], scr[:], start=True, stop=True,
                )
            for tt in range(1, NTT):
                nc.sync.dma_start(
                    xt3[:, :, tt * 512:(tt + 1) * 512],
                    xt_d[:, tt * 512:(tt + 1) * 512].rearrange(
                        "(c p) t -> p c t", p=128
                    ),
                )
            wo_sb = cst.tile([128, NHP * C], BF16, tag="wo")
            wo3 = wo_sb[:].rearrange("p (h f) -> p h f", h=NHP)
            nc.sync.dma_start(wo3, wo_d[:, :, :].rearrange("h p f -> p h f"))

            # ---------------- fine-grained emission units ----------------
            # Each filler unit is (pe_ns_estimate, closure). The scheduler pops
            # fillers to cover the Act-vs-PE deficit inside attention units.
            def u_qk(tt, fb, w3, b_sb, dst3, nm):
                key = (nm, tt, fb)
                def go():
                    ps = mp.tile([128, 512], F32, tag="mp", name=f"{nm}{tt}_{fb}")
                    for cc in range(NCC):
                        nc.tensor.matmul(
                            ps[:],
                            w3[:, cc, fb * 128:(fb + 1) * 128],
                            xt3[:, cc, tt * 512:(tt + 1) * 512],
                            start=(cc == 0),
                            stop=(cc == NCC - 1),
                        )
                    nc.vector.tensor_scalar_add(
                        dst3[:, fb, tt * 512:(tt + 1) * 512], ps[:],
                        b_sb[:, fb:fb + 1],
                    )
                return (key, 1707, go)

            def u_v(tb):
                key = ("vt", tb)
                def go():
                    ps = mp.tile([128, 512], F32, tag="mp", name=f"v{tb}")
                    for cc in range(NCC):
                        nc.tensor.matmul(
                            ps[:],
                            xt3[:, cc, tb * 128:(tb + 1) * 128],
                            wv3[:, cc, :],
                            start=(cc == 0),
                            stop=(cc == NCC - 1),
                        )
                    nc.vector.tensor_copy(
                        v4[:, tb, :, 0:D],
                        ps[:].rearrange("p (h d) -> p h d", h=NHL),
                    )
                return (key, 1707, go)

            def u_op(tb, ct, on_act=False):
                key = ("op", tb, ct)
                def go():
                    zps = mp.tile([128, 512], F32, tag="mp", name=f"z{tb}_{ct}")
                    for cc in range(NHP):
                        nc.tensor.matmul(
                            zps[:],
                            yt3[:, cc, tb * 128:(tb + 1) * 128],
                            wo3[:, cc, ct * 512:(ct + 1) * 512],
                            start=(cc == 0),
                            stop=(cc == NHP - 1),
                        )
                    z_sb = zb.tile([128, 512], F32, tag="z", name=f"zs{tb}_{ct}")
                    if on_act:
                        nc.scalar.copy(z_sb[:], zps[:])
                    else:
                        nc.vector.tensor_copy(z_sb[:], zps[:])
                    nc.sync.dma_start(
                        zpart[ZROW[tb]:ZROW[tb] + 128, ct * 512:(ct + 1) * 512],
                        z_sb[:],
                    )
                return (key, 880, go)

            fillers = []          # deque of (key, pe_ns, closure)
            done_keys = set()
            deficit = [0.0]       # act-over-pe backlog inside attention

            def fill(need_ns):
                deficit[0] += need_ns
                while fillers and deficit[0] > fillers[0][1] * 2.0:
                    key, pe_ns, go = fillers.pop(0)
                    go()
                    done_keys.add(key)
                    deficit[0] -= pe_ns

            def require(*keys):
                # run exactly the queued fillers the caller depends on
                need = {k for k in keys if k not in done_keys}
                if not need:
                    return
                rest = []
                forced = []
                for key, pe_ns, go in fillers:
                    if key in need:
                        go()
                        done_keys.add(key)
                        deficit[0] -= pe_ns
                        need.discard(key)
                        forced.append(key)
                    else:
                        rest.append((key, pe_ns, go))
                fillers[:] = rest

                if need:
                    raise RuntimeError(f"missing filler deps: {need}")

            pend_tp = []          # pending y-transpose closures (pair-complete)

            def flush_tp(keep=0):
                while len(pend_tp) > keep:
                    pend_tp.pop(0)()

            ysave = {}

            def emit_att(h, qt):
                hp, hi = h // 2, h % 2
                n_kb = 4 * (qt + 1)
                require(("q", qt, hp),
                        *[("k", t, hp) for t in range(qt + 1)],
                        *[("vt", tb) for tb in range(4 * qt + 4)])
                flush_tp(keep=2)
                # one psum bank; a single accumulation group brackets all four
                # qb sub-regions (start marks the bank pending-zero, first
                # write per byte overwrites, later writes accumulate)
                Y = yp.tile([128, 512], F32, tag="y", name=f"Y{h}_{qt}")
                Y3 = Y[:, 0:4 * VE].rearrange("p (q e) -> p q e", q=4)
                for kbp in range(n_kb // 2):
                    s = sp.tile([128, 1024], F32, tag="s", name=f"s{h}_{qt}_{kbp}")
                    cs = []
                    pe_ns = 0.0
                    for c2 in range(2):
                        kb = 2 * kbp + c2
                        c = kb - 4 * qt
                        cs.append(c)
                        j0 = c * 128 if c > 0 else 0
                        nc.tensor.matmul(
                            s[:, c2 * 512 + j0:(c2 + 1) * 512],
                            kt3[hi * 64:(hi + 1) * 64, hp, kb * 128:(kb + 1) * 128],
                            qt3[hi * 64:(hi + 1) * 64, hp, qt * 512 + j0:(qt + 1) * 512],
                            tile_position=(hi * 64, 0),
                            start=True,
                            stop=not (0 <= c <= 3),
                        )
                        pe_ns += (512 - j0) * 0.4167
                        if 0 <= c <= 3:
                            nc.tensor.matmul(
                                s[:, c2 * 512 + c * 128: c2 * 512 + (c + 1) * 128],
                                id_sb[:],
                                neg_sb[:],
                                start=False,
                                stop=True,
                            )
                            pe_ns += 128 * 0.4167
                    P = pp.tile([128, 1024], BF16, tag="p", name=f"p{h}_{qt}_{kbp}")
                    if 0 <= cs[0] <= 3:
                        # diagonal pair: exp the two written ranges, skipping
                        # the never-written fully-masked hole between blocks
                        ja = cs[0] * 128
                        jb = 512 + cs[1] * 128
                        nc.scalar.activation(
                            P[:, ja:512], s[:, ja:512], AF.Exp, scale=0.125
                        )
                        nc.scalar.activation(
                            P[:, jb:], s[:, jb:], AF.Exp, scale=0.125
                        )
                        act_ns = (1536 - ja - jb) * 0.833 + 300
                    else:
                        nc.scalar.activation(P[:], s[:], AF.Exp, scale=0.125)
                        act_ns = 1024 * 0.833 + 150
                    # filler lands between the scores and AV matmuls so the
                    # PE covers the exp latency (in-order execution); the AV
                    # rows still count toward this kbp's PE supply
                    n_av = sum(1 for c2_ in range(2) for qb_ in range(4)
                               if (2 * kbp + c2_ - 4 * qt) <= qb_)
                    fill(max(0.0, act_ns - pe_ns - n_av * VE * 0.4167))
                    for c2 in range(2):
                        kb = 2 * kbp + c2
                        c = kb - 4 * qt
                        for qb in range(4):
                            if c > qb:
                                continue
                            nc.tensor.matmul(
                                Y3[:, qb, :],
                                P[:, c2 * 512 + qb * 128: c2 * 512 + (qb + 1) * 128],
                                v4[:, kb, h, :],
                                start=(kb == 0 and qb == 0),
                                stop=(kb == n_kb - 1 and qb == 3),
                            )
                # normalize: y = Y[:, :64] / Y[:, 64]
                r = rc.tile([128, 4], F32, tag="r", name=f"r{h}_{qt}")
                nc.vector.reciprocal(
                    r[:], Y3[:, :, D:D + 1].rearrange("p q e -> p (q e)")
                )
                y = yb.tile([128, 4 * D], BF16, tag="yb", name=f"y{h}_{qt}")
                y3 = y[:].rearrange("p (q d) -> p q d", q=4)
                for qb in range(4):
                    nc.vector.tensor_scalar(
                        y3[:, qb, :], Y3[:, qb, 0:D], r[:, qb:qb + 1], None,
                        mybir.AluOpType.mult,
                    )
                if hi == 0:
                    ysave[(h, qt)] = y
                else:
                    ye = ysave.pop((h - 1, qt))

                    def tp_go(ye=ye, y=y, hp=hp, qt=qt):
                        ye3 = ye[:].rearrange("p (q d) -> p q d", q=4)
                        yo3 = y[:].rearrange("p (q d) -> p q d", q=4)
                        tp = sp.tile([128, 2048], BF16, tag="s", name=f"yt{hp}_{qt}")
                        for qb in range(4):
                            nc.tensor.transpose(
                                tp[0:64, qb * 128:(qb + 1) * 128], ye3[:, qb, :],
                                id_sb[:],
                            )
                            nc.tensor.transpose(
                                tp[64:128, qb * 128:(qb + 1) * 128], yo3[:, qb, :],
                                id_sb[:],
                            )
                        nc.vector.tensor_copy(
                            yt3[:, hp, qt * 512:(qt + 1) * 512], tp[:, 0:512]
                        )
                    pend_tp.append(tp_go)

            def emit_rs(c):
                nc.gpsimd.collective_compute(
                    "ReduceScatter",
                    mybir.AluOpType.add,
                    replica_groups=[[0, 1], [2, 3], [4, 5], [6, 7]],
                    ins=[zpart[c * 256:(c + 1) * 256, :].opt()],
                    outs=[zreds[c].opt()],
                )
                nc.sync.dma_start(zh[c * 128:(c + 1) * 128, :], zreds[c][:])

            # ---------------- static schedule ----------------
            def push(*units):
                fillers.extend(units)

            def run_now(*units):
                for key, _, go in units:
                    go()
                    done_keys.add(key)

            # K/V/Q for tt0 emitted directly (attention qt0 needs them)
            run_now(*[u_qk(0, fb, wk3, bk_sb, kt3, "k") for fb in range(NHP)])
            run_now(*[u_v(tb) for tb in range(4)])
            run_now(*[u_qk(0, fb, wq3, bq_sb, qt3, "q") for fb in range(NHP)])

            # remaining QKV work becomes filler, in dependency-safe order
            push(*[u_qk(1, fb, wk3, bk_sb, kt3, "k") for fb in range(NHP)])
            push(*[u_v(tb) for tb in range(4, 8)])
            push(*[u_qk(1, fb, wq3, bq_sb, qt3, "q") for fb in range(NHP)])
            push(*[u_qk(2, fb, wk3, bk_sb, kt3, "k") for fb in range(NHP)])
            push(*[u_v(tb) for tb in range(8, 12)])
            push(*[u_qk(2, fb, wq3, bq_sb, qt3, "q") for fb in range(NHP)])

            for h in range(NHL):
                emit_att(h, 0)
            # qt0 done -> out-proj tb0-3 available; also prepare tt3 inputs
            push(*[u_qk(3, fb, wk3, bk_sb, kt3, "k") for fb in range(NHP)])
            push(*[u_v(tb) for tb in range(12, 16)])
            push(*[u_qk(3, fb, wq3, bq_sb, qt3, "q") for fb in range(NHP)])
            flush_tp()

            for h in range(NHL):
                emit_att(h, 1)
            flush_tp()
            push(*[u_op(tb, ct) for tb in range(0, 8) for ct in range(2)])

            # interleave qt2 and qt3 heads; qt3 needs K/V/Q(tt3) which sit at
            # the front of the filler queue by now
            lead = [(0, 2), (1, 2)]
            tail_o = [(6, 3), (7, 3)]
            mid = []
            for i in range(NHL - 2):
                mid.append((i, 3))
                mid.append((i + 2, 2))
            order = lead + mid + tail_o
            for h, qt in order:
                emit_att(h, qt)
            flush_tp()
            push(*[u_op(tb, ct, on_act=(ct == 1)) for tb in range(8, 12) for ct in range(2)])

            # drain remaining fillers (anything the deficit model didn't pull)
            while fillers:
                fillers.pop(0)[2]()
            for c in range(4):
                emit_rs(c)
            for tb in range(12, 16):
                # drain-region out-proj: both halves in one wide s-pool psum
                # tile, copies split across DVE and Act so they overlap
                zps = sp.tile([128, 1024], F32, tag="s", name=f"zw{tb}")
                for ct in range(2):
                    for cc in range(NHP):
                        nc.tensor.matmul(
                            zps[:, ct * 512:(ct + 1) * 512],
                            yt3[:, cc, tb * 128:(tb + 1) * 128],
                            wo3[:, cc, ct * 512:(ct + 1) * 512],
                            start=(cc == 0),
                            stop=(cc == NHP - 1),
                        )
                z_sb = zb.tile([128, 1024], F32, tag="zw", name=f"zsw{tb}")
                nc.vector.tensor_copy(z_sb[:, 0:512], zps[:, 0:512])
                nc.sync.dma_start(
                    zpart[ZROW[tb]:ZROW[tb] + 128, 0:512], z_sb[:, 0:512]
                )
                nc.scalar.copy(z_sb[:, 512:1024], zps[:, 512:1024])
                nc.sync.dma_start(
                    zpart[ZROW[tb]:ZROW[tb] + 128, 512:1024], z_sb[:, 512:1024]
                )
                emit_rs(tb - 8)

    nc.compile()
    return nc


_NC_CACHE = None


def _get_nc():
    global _NC_CACHE
    if _NC_CACHE is None:
        _NC_CACHE = build()
    return _NC_CACHE


def _bf16(a):
    import ml_dtypes
    return np.ascontiguousarray(a).astype(ml_dtypes.bfloat16)


def _in_maps(x, Wqkv, bqkv, Wo, bo):
    x = np.asarray(x, dtype=np.float32)
    Wqkv = np.asarray(Wqkv, dtype=np.float32)
    bqkv = np.asarray(bqkv, dtype=np.float32)
    Wo = np.asarray(Wo, dtype=np.float32)
    bo = np.asarray(bo, dtype=np.float32)

    ident = np.eye(128, dtype=np.float32)
    i_ = np.arange(128)[:, None]
    j_ = np.arange(128)[None, :]
    negtri = np.where(i_ > j_, np.float32(NEG), np.float32(0.0))

    in_maps = []
    for core in range(8):
        b, hh = core // 2, core % 2
        sl = slice(hh * FL, (hh + 1) * FL)
        wq = Wqkv[:, 0 * C:1 * C][:, sl]
        wk = Wqkv[:, 1 * C:2 * C][:, sl]
        wv = Wqkv[:, 2 * C:3 * C][:, sl]
        wo = Wo[sl, :]
        in_maps.append({
            "xt": _bf16(x[b].T),
            "wq": _bf16(wq.reshape(NCC, 128, FL)),
            "wk": _bf16(wk.reshape(NCC, 128, FL)),
            "wv": _bf16(wv.reshape(NCC, 128, FL)),
            "wo": _bf16(wo.reshape(NHP, 128, C)),
            "bq": np.ascontiguousarray(bqkv[0 * C:1 * C][sl].reshape(NHP, 128).T),
            "bk": np.ascontiguousarray(bqkv[1 * C:2 * C][sl].reshape(NHP, 128).T),
            "ident": _bf16(ident),
            "negtri": _bf16(negtri),
            "identr": ident,
            "vones": _bf16(np.ones((128, NTB * NHL), dtype=np.float32)),
        })
    return in_maps


def _assemble(res, bias):
    out = np.empty((B, T, C), dtype=np.float32)
    for b in range(B):
        out[b, : T // 2] = res.results[2 * b]["zh"]
        out[b, T // 2:] = res.results[2 * b + 1]["zh"]
    out += bias[None, None, :]
    return out


def kernel(x, Wqkv, bqkv, Wo, bo):
    in_maps = _in_maps(x, Wqkv, bqkv, Wo, bo)
    res = run_bass_kernel_spmd(_get_nc(), in_maps, core_ids=list(range(8)))
    # v-bias and output bias fold to a constant vector: softmax weights sum to
    # 1, so y = P@(v+bv)/rs = P@v/rs + bv  ->  out += bo + bv @ Wo
    bqkv_ = np.asarray(bqkv, dtype=np.float32)
    bias = (np.asarray(bo, dtype=np.float32)
            + bqkv_[2 * C:3 * C] @ np.asarray(Wo, dtype=np.float32))
    return _assemble(res, bias)


def run_traced(x, Wqkv, bqkv, Wo, bo, trace_cores=None):
    in_maps = _in_maps(x, Wqkv, bqkv, Wo, bo)
    res = run_bass_kernel_spmd(
        _get_nc(), in_maps, core_ids=list(range(8)), trace=True,
        trace_cores=trace_cores,
    )
    return res


# revision 4
# speedup vs baseline: 1.0076x; 1.0012x over previous
"""Causal multi-head attention block (QKV proj -> causal MHA -> out proj) on 8 Trainium2
cores — v3.

Sharding: core = b*2 + hh handles batch b (of 4) and head-half hh (8 of 16 heads).
Host pre-transposes x (x^T) and pre-formats all weights as bf16, so the device does
only GEMMs + attention. All matmuls run in bf16 (fp32 PSUM accumulation); softmax is
fp32 on the Activation engine.

Structure per core:
  - Q^T/K^T [feat(part), tok] and V [tok(part), feat] projections from resident x^T.
  - scores^T tiles [k(part), q] with the causal mask added via an extra accumulate
    matmul (identity x neg-triangle) on the PE, then exp on Act into bf16 P tiles.
  - AV with P stationary: Y[q(part), 65] accumulated over k-blocks; col 64 is the
    softmax denominator via a ones-column in V. Normalize = DVE reciprocal +
    per-partition tensor_scalar multiply; y^T via PE transposes; out-proj into
    token-major z with a pairwise ReduceScatter.
  - The Act engine runs only exp and is the secondary bottleneck; attention units
    are interleaved with fine-grained QKV / out-proj filler via a deficit model so
    the PE never waits on exp.
"""

import numpy as np

import concourse.bass as bass
import concourse.tile as tile
from concourse import bacc, mybir
from concourse.bass_utils import run_bass_kernel_spmd

F32 = mybir.dt.float32
F32R = mybir.dt.float32r
BF16 = mybir.dt.bfloat16
AF = mybir.ActivationFunctionType

B, T, C, H = 4, 2048, 1024, 16
D = C // H          # 64
NHL = H // 2        # 8 local heads per core
NHP = NHL // 2      # 4 local head pairs
FL = NHL * D        # 512 local features
NCC = C // 128      # 8 contraction chunks over C
NTB = T // 128      # 16 token blocks
NTT = T // 512      # 4 token tiles / qtiles
VE = D + 1          # 65: V feature cols + ones column
NEG = -1.0e30

# zpart row remap so each pairwise-RS chunk is a contiguous 256-row block:
# chunk c holds tb c (rank0 tokens) then tb 8+c (rank1 tokens)
ZROW = {}
for _c in range(8):
    ZROW[_c] = _c * 256
    ZROW[8 + _c] = _c * 256 + 128


def build():
    nc = bacc.Bacc("TRN2", target_bir_lowering=False, num_devices=8)

    xt_d = nc.dram_tensor("xt", [C, T], BF16, kind="ExternalInput")
    wq_d = nc.dram_tensor("wq", [NCC, 128, FL], BF16, kind="ExternalInput")
    wk_d = nc.dram_tensor("wk", [NCC, 128, FL], BF16, kind="ExternalInput")
    wv_d = nc.dram_tensor("wv", [NCC, 128, FL], BF16, kind="ExternalInput")
    wo_d = nc.dram_tensor("wo", [NHP, 128, C], BF16, kind="ExternalInput")
    bq_d = nc.dram_tensor("bq", [128, NHP], F32, kind="ExternalInput")
    bk_d = nc.dram_tensor("bk", [128, NHP], F32, kind="ExternalInput")
    id_d = nc.dram_tensor("ident", [128, 128], BF16, kind="ExternalInput")
    neg_d = nc.dram_tensor("negtri", [128, 128], BF16, kind="ExternalInput")
    idr_d = nc.dram_tensor("identr", [128, 128], F32R, kind="ExternalInput")
    ones_d = nc.dram_tensor("vones", [128, NTB * NHL], BF16, kind="ExternalInput")
    zh = nc.dram_tensor("zh", [T // 2, C], F32, kind="ExternalOutput")

    with tile.TileContext(nc) as tc:
        with (
            tc.tile_pool(name="res", bufs=1) as res,
            tc.tile_pool(name="cst", bufs=1) as cst,
            tc.tile_pool(name="dram", bufs=1, space="DRAM") as dram,
            tc.tile_pool(name="mp", bufs=2, space="PSUM") as mp,       # 2 banks
            tc.tile_pool(name="sp", bufs=2, space="PSUM") as sp,       # 4 banks
            tc.tile_pool(name="yp", bufs=2, space="PSUM") as yp,       # 2 banks
            tc.tile_pool(name="pp", bufs=8) as pp,
            tc.tile_pool(name="yb", bufs=6) as yb,
            tc.tile_pool(name="rc", bufs=6) as rc,
            tc.tile_pool(name="zb", bufs=3) as zb,
        ):
            # ---------------- resident tiles ----------------
            xt = res.tile([128, NCC * T], BF16)          # x^T  [c | cc, tok]
            xt3 = xt[:].rearrange("p (c t) -> p c t", c=NCC)
            qt_sb = res.tile([128, NHP * T], BF16)       # Q^T  [d(2 heads) | hp, tok]
            qt3 = qt_sb[:].rearrange("p (h t) -> p h t", h=NHP)
            kt_sb = res.tile([128, NHP * T], BF16)       # K^T
            kt3 = kt_sb[:].rearrange("p (h t) -> p h t", h=NHP)
            v_sb = res.tile([128, NTB * NHL * VE], BF16)  # V' [k | tb, h, d+1]
            v4 = v_sb[:].rearrange("p (b h e) -> p b h e", b=NTB, h=NHL)
            yt_sb = res.tile([128, NHP * T], BF16)       # y^T [f(2 heads) | hp, tok]
            yt3 = yt_sb[:].rearrange("p (h t) -> p h t", h=NHP)

            zpart = dram.tile([T, C], F32, name="zpart")
            zreds = [dram.tile([128, C], F32, name=f"zred{i}") for i in range(8)]

            # ------------- loads, ordered so K(tt0)/V(tt0) unblock fastest -------------
            scr = cst.tile([128, 128], BF16, tag="scr")
            nc.vector.memset(scr[:], 0.0)
            nc.sync.dma_start(
                xt3[:, :, 0:512],
                xt_d[:, 0:512].rearrange("(c p) t -> p c t", p=128),
            )
            wk_sb = cst.tile([128, NCC * FL], BF16, tag="wk")
            wk3 = wk_sb[:].rearrange("p (c f) -> p c f", c=NCC)
            nc.sync.dma_start(
                wk3[:, :, 0:256], wk_d[:, :, 0:256].rearrange("c p f -> p c f")
            )
            bk_sb = cst.tile([128, NHP], F32, tag="bk")
            nc.sync.dma_start(bk_sb[:], bk_d[:, :])
            nc.sync.dma_start(
                wk3[:, :, 256:512], wk_d[:, :, 256:512].rearrange("c p f -> p c f")
            )
            id_sb = cst.tile([128, 128], BF16, tag="id")
            nc.sync.dma_start(id_sb[:], id_d[:, :])
            neg_sb = cst.tile([128, 128], BF16, tag="neg")
            nc.sync.dma_start(neg_sb[:], neg_d[:, :])
            idr_sb = cst.tile([128, 128], F32R, tag="idr")
            nc.sync.dma_start(idr_sb[:], idr_d[:, :])
            bq_sb = cst.tile([128, NHP], F32, tag="bq")
            nc.sync.dma_start(bq_sb[:], bq_d[:, :])
            wv_sb = cst.tile([128, NCC * FL], BF16, tag="wv")
            wv3 = wv_sb[:].rearrange("p (c f) -> p c f", c=NCC)
            nc.sync.dma_start(wv3, wv_d[:, :, :].rearrange("c p f -> p c f"))
            wq_sb = cst.tile([128, NCC * FL], BF16, tag="wq")
            wq3 = wq_sb[:].rearrange("p (c f) -> p c f", c=NCC)
            nc.sync.dma_start(wq3, wq_d[:, :, :].rearrange("c p f -> p c f"))
            # warm the exp table set (hides ACT_TABLE_LOAD)
            warm = cst.tile([1, 1], F32, tag="warm")
            nc.scalar.activation(warm[:], bq_sb[0:1, 0:1], AF.Exp)
            # ones column of V' via a small DMA + one strided DVE copy
            ones_sb = cst.tile([128, NTB * NHL], BF16, tag="ones")
            nc.sync.dma_start(ones_sb[:], ones_d[:, :])
            nc.vector.tensor_copy(
                v4[:, :, :, D:D + 1],
                ones_sb[:].rearrange("p (b h) -> p b h", b=NTB).unsqueeze(3),
            )
            # PE p-state warm-up while the x^T/weight DMAs land (scratch is
            # memset-initialized, so no DMA gates the first matmul)
            wps = sp.tile([128, 1024], F32, tag="s", name="warmps")
            for wi in range(80):
                nc.tensor.matmul(
                    wps[:, (wi % 8) * 128:(wi % 8) * 128 + 128],
                    scr[:], scr[:], start=True, stop=True,
                )
            for tt in range(1, NTT):
                nc.sync.dma_start(
                    xt3[:, :, tt * 512:(tt + 1) * 512],
                    xt_d[:, tt * 512:(tt + 1) * 512].rearrange(
                        "(c p) t -> p c t", p=128
                    ),
                )
            wo_sb = cst.tile([128, NHP * C], BF16, tag="wo")
            wo3 = wo_sb[:].rearrange("p (h f) -> p h f", h=NHP)
            nc.sync.dma_start(wo3, wo_d[:, :, :].rearrange("h p f -> p h f"))

            # ---------------- fine-grained emission units ----------------
            # Each filler unit is (pe_ns_estimate, closure). The scheduler pops
            # fillers to cover the Act-vs-PE deficit inside attention units.
            def u_qk(tt, fb, w3, b_sb, dst3, nm):
                key = (nm, tt, fb)
                def go():
                    ps = mp.tile([128, 512], F32, tag="mp", name=f"{nm}{tt}_{fb}")
                    for cc in range(NCC):
                        nc.tensor.matmul(
                            ps[:],
                            w3[:, cc, fb * 128:(fb + 1) * 128],
                            xt3[:, cc, tt * 512:(tt + 1) * 512],
                            start=(cc == 0),
                            stop=(cc == NCC - 1),
                        )
                    nc.vector.tensor_scalar_add(
                        dst3[:, fb, tt * 512:(tt + 1) * 512], ps[:],
                        b_sb[:, fb:fb + 1],
                    )
                return (key, 1707, go)

            def u_v(tb):
                key = ("vt", tb)
                def go():
                    ps = mp.tile([128, 512], F32, tag="mp", name=f"v{tb}")
                    for cc in range(NCC):
                        nc.tensor.matmul(
                            ps[:],
                            xt3[:, cc, tb * 128:(tb + 1) * 128],
                            wv3[:, cc, :],
                            start=(cc == 0),
                            stop=(cc == NCC - 1),
                        )
                    nc.vector.tensor_copy(
                        v4[:, tb, :, 0:D],
                        ps[:].rearrange("p (h d) -> p h d", h=NHL),
                    )
                return (key, 1707, go)

            def u_op(tb, ct, on_act=False):
                key = ("op", tb, ct)
                def go():
                    zps = mp.tile([128, 512], F32, tag="mp", name=f"z{tb}_{ct}")
                    for cc in range(NHP):
                        nc.tensor.matmul(
                            zps[:],
                            yt3[:, cc, tb * 128:(tb + 1) * 128],
                            wo3[:, cc, ct * 512:(ct + 1) * 512],
                            start=(cc == 0),
                            stop=(cc == NHP - 1),
                        )
                    z_sb = zb.tile([128, 512], F32, tag="z", name=f"zs{tb}_{ct}")
                    if on_act:
                        nc.scalar.copy(z_sb[:], zps[:])
                    else:
                        nc.vector.tensor_copy(z_sb[:], zps[:])
                    nc.sync.dma_start(
                        zpart[ZROW[tb]:ZROW[tb] + 128, ct * 512:(ct + 1) * 512],
                        z_sb[:],
                    )
                return (key, 880, go)

            fillers = []          # deque of (key, pe_ns, closure)
            done_keys = set()
            deficit = [0.0]       # act-over-pe backlog inside attention

            def fill(need_ns):
                deficit[0] += need_ns
                while fillers and deficit[0] > fillers[0][1] * 1.5:
                    key, pe_ns, go = fillers.pop(0)
                    go()
                    done_keys.add(key)
                    deficit[0] -= pe_ns

            def require(*keys):
                # run exactly the queued fillers the caller depends on
                need = {k for k in keys if k not in done_keys}
                if not need:
                    return
                rest = []
                forced = []
                for key, pe_ns, go in fillers:
                    if key in need:
                        go()
                        done_keys.add(key)
                        deficit[0] -= pe_ns
                        need.discard(key)
                        forced.append(key)
                    else:
                        rest.append((key, pe_ns, go))
                fillers[:] = rest

                if need:
                    raise RuntimeError(f"missing filler deps: {need}")

            pend_tp = []          # pending y-transpose closures (pair-complete)

            def flush_tp(keep=0):
                while len(pend_tp) > keep:
                    pend_tp.pop(0)()

            def emit_att_pair(he, qt):
                # both heads of a pair interleaved at kb-pair granularity:
                # head B's scores cover head A's exp latency and vice versa
                hp = he // 2
                n_kb = 4 * (qt + 1)
                require(("q", qt, hp),
                        *[("k", t, hp) for t in range(qt + 1)],
                        *[("vt", tb) for tb in range(4 * qt + 4)])
                flush_tp(keep=2)
                Ys = [yp.tile([128, 512], F32, tag="y", name=f"Y{he + i}_{qt}")
                      for i in range(2)]
                Y3s = [Y[:, 0:4 * VE].rearrange("p (q e) -> p q e", q=4)
                       for Y in Ys]
                for kbp in range(n_kb // 2):
                    Ps = []
                    css = None
                    act_ns = 0.0
                    pe_ns = 0.0
                    for hi in range(2):
                        s = sp.tile([128, 1024], F32, tag="s",
                                    name=f"s{he + hi}_{qt}_{kbp}")
                        cs = []
                        for c2 in range(2):
                            kb = 2 * kbp + c2
                            c = kb - 4 * qt
                            cs.append(c)
                            j0 = c * 128 if c > 0 else 0
                            nc.tensor.matmul(
                                s[:, c2 * 512 + j0:(c2 + 1) * 512],
                                kt3[hi * 64:(hi + 1) * 64, hp,
                                    kb * 128:(kb + 1) * 128],
                                qt3[hi * 64:(hi + 1) * 64, hp,
                                    qt * 512 + j0:(qt + 1) * 512],
                                tile_position=(hi * 64, 0),
                                start=True,
                                stop=not (0 <= c <= 3),
                            )
                            pe_ns += (512 - j0) * 0.4167
                            if 0 <= c <= 3:
                                nc.tensor.matmul(
                                    s[:, c2 * 512 + c * 128:
                                      c2 * 512 + (c + 1) * 128],
                                    id_sb[:],
                                    neg_sb[:],
                                    start=False,
                                    stop=True,
                                )
                                pe_ns += 128 * 0.4167
                        css = cs
                        P = pp.tile([128, 1024], BF16, tag="p",
                                    name=f"p{he + hi}_{qt}_{kbp}")
                        if 0 <= cs[0] <= 3:
                            ja = cs[0] * 128
                            jb = 512 + cs[1] * 128
                            nc.scalar.activation(
                                P[:, ja:512], s[:, ja:512], AF.Exp, scale=0.125
                            )
                            nc.scalar.activation(
                                P[:, jb:], s[:, jb:], AF.Exp, scale=0.125
                            )
                            act_ns += (1536 - ja - jb) * 0.833 + 300
                        else:
                            nc.scalar.activation(P[:], s[:], AF.Exp, scale=0.125)
                            act_ns += 1024 * 0.833 + 150
                        Ps.append(P)
                    n_av = sum(1 for c2_ in range(2) for qb_ in range(4)
                               if (2 * kbp + c2_ - 4 * qt) <= qb_)
                    fill(max(0.0, act_ns - pe_ns - 2 * n_av * VE * 0.4167))
                    for hi in range(2):
                        for c2 in range(2):
                            kb = 2 * kbp + c2
                            c = kb - 4 * qt
                            for qb in range(4):
                                if c > qb:
                                    continue
                                nc.tensor.matmul(
                                    Y3s[hi][:, qb, :],
                                    Ps[hi][:, c2 * 512 + qb * 128:
                                           c2 * 512 + (qb + 1) * 128],
                                    v4[:, kb, he + hi, :],
                                    start=(kb == 0 and qb == 0),
                                    stop=(kb == n_kb - 1 and qb == 3),
                                )
                # normalize both heads, then defer the pair transpose
                ys = []
                for hi in range(2):
                    r = rc.tile([128, 4], F32, tag="r", name=f"r{he + hi}_{qt}")
                    nc.vector.reciprocal(
                        r[:], Y3s[hi][:, :, D:D + 1].rearrange("p q e -> p (q e)")
                    )
                    y = yb.tile([128, 4 * D], BF16, tag="yb",
                                name=f"y{he + hi}_{qt}")
                    y3 = y[:].rearrange("p (q d) -> p q d", q=4)
                    for qb in range(4):
                        nc.vector.tensor_scalar(
                            y3[:, qb, :], Y3s[hi][:, qb, 0:D], r[:, qb:qb + 1],
                            None, mybir.AluOpType.mult,
                        )
                    ys.append(y)

                def tp_go(ys=ys, hp=hp, qt=qt):
                    tp = sp.tile([128, 2048], BF16, tag="s", name=f"yt{hp}_{qt}")
                    for hi in range(2):
                        y3 = ys[hi][:].rearrange("p (q d) -> p q d", q=4)
                        for qb in range(4):
                            nc.tensor.transpose(
                                tp[hi * 64:(hi + 1) * 64,
                                   qb * 128:(qb + 1) * 128],
                                y3[:, qb, :], id_sb[:],
                            )
                    nc.vector.tensor_copy(
                        yt3[:, hp, qt * 512:(qt + 1) * 512], tp[:, 0:512]
                    )
                pend_tp.append(tp_go)

            def emit_rs(c):
                nc.gpsimd.collective_compute(
                    "ReduceScatter",
                    mybir.AluOpType.add,
                    replica_groups=[[0, 1], [2, 3], [4, 5], [6, 7]],
                    ins=[zpart[c * 256:(c + 1) * 256, :].opt()],
                    outs=[zreds[c].opt()],
                )
                nc.sync.dma_start(zh[c * 128:(c + 1) * 128, :], zreds[c][:])

            # ---------------- static schedule ----------------
            def push(*units):
                fillers.extend(units)

            def run_now(*units):
                for key, _, go in units:
                    go()
                    done_keys.add(key)

            # K/V/Q for tt0 emitted directly (attention qt0 needs them)
            run_now(*[u_qk(0, fb, wk3, bk_sb, kt3, "k") for fb in range(NHP)])
            run_now(*[u_v(tb) for tb in range(4)])
            run_now(*[u_qk(0, fb, wq3, bq_sb, qt3, "q") for fb in range(NHP)])

            push(*[u_qk(1, fb, wk3, bk_sb, kt3, "k") for fb in range(NHP)])
            push(*[u_v(tb) for tb in range(4, 8)])
            push(*[u_qk(1, fb, wq3, bq_sb, qt3, "q") for fb in range(NHP)])
            push(*[u_qk(2, fb, wk3, bk_sb, kt3, "k") for fb in range(NHP)])
            push(*[u_v(tb) for tb in range(8, 12)])
            push(*[u_qk(2, fb, wq3, bq_sb, qt3, "q") for fb in range(NHP)])

            for he in range(0, NHL, 2):
                emit_att_pair(he, 0)
            push(*[u_qk(3, fb, wk3, bk_sb, kt3, "k") for fb in range(NHP)])
            push(*[u_v(tb) for tb in range(12, 16)])
            push(*[u_qk(3, fb, wq3, bq_sb, qt3, "q") for fb in range(NHP)])
            flush_tp()

            for he in range(0, NHL, 2):
                emit_att_pair(he, 1)
            flush_tp()
            push(*[u_op(tb, ct) for tb in range(0, 8) for ct in range(2)])

            order = [(0, 2), (0, 3), (2, 3), (2, 2), (4, 3), (4, 2),
                     (6, 3), (6, 2)]
            for he, qt in order:
                emit_att_pair(he, qt)
            flush_tp()
            push(*[u_op(tb, ct, on_act=(ct == 1)) for tb in range(8, 12)
                   for ct in range(2)])

            # drain remaining fillers (anything the deficit model didn't pull)
            while fillers:
                fillers.pop(0)[2]()
            for c in range(4):
                emit_rs(c)
            for tb in range(12, 16):
                # drain-region out-proj: both halves in one wide s-pool psum
                # tile, copies split across DVE and Act so they overlap
                zps = sp.tile([128, 1024], F32, tag="s", name=f"zw{tb}")
                for ct in range(2):
                    for cc in range(NHP):
                        nc.tensor.matmul(
                            zps[:, ct * 512:(ct + 1) * 512],
                            yt3[:, cc, tb * 128:(tb + 1) * 128],
                            wo3[:, cc, ct * 512:(ct + 1) * 512],
                            start=(cc == 0),
                            stop=(cc == NHP - 1),
                        )
                z_sb = zb.tile([128, 1024], F32, tag="zw", name=f"zsw{tb}")
                nc.vector.tensor_copy(z_sb[:, 0:512], zps[:, 0:512])
                nc.sync.dma_start(
                    zpart[ZROW[tb]:ZROW[tb] + 128, 0:512], z_sb[:, 0:512]
                )
                nc.scalar.copy(z_sb[:, 512:1024], zps[:, 512:1024])
                nc.sync.dma_start(
                    zpart[ZROW[tb]:ZROW[tb] + 128, 512:1024], z_sb[:, 512:1024]
                )
                emit_rs(tb - 8)

    nc.compile()
    return nc


_NC_CACHE = None


def _get_nc():
    global _NC_CACHE
    if _NC_CACHE is None:
        _NC_CACHE = build()
    return _NC_CACHE


def _bf16(a):
    import ml_dtypes
    return np.ascontiguousarray(a).astype(ml_dtypes.bfloat16)


def _in_maps(x, Wqkv, bqkv, Wo, bo):
    x = np.asarray(x, dtype=np.float32)
    Wqkv = np.asarray(Wqkv, dtype=np.float32)
    bqkv = np.asarray(bqkv, dtype=np.float32)
    Wo = np.asarray(Wo, dtype=np.float32)
    bo = np.asarray(bo, dtype=np.float32)

    ident = np.eye(128, dtype=np.float32)
    i_ = np.arange(128)[:, None]
    j_ = np.arange(128)[None, :]
    negtri = np.where(i_ > j_, np.float32(NEG), np.float32(0.0))

    in_maps = []
    for core in range(8):
        b, hh = core // 2, core % 2
        sl = slice(hh * FL, (hh + 1) * FL)
        wq = Wqkv[:, 0 * C:1 * C][:, sl]
        wk = Wqkv[:, 1 * C:2 * C][:, sl]
        wv = Wqkv[:, 2 * C:3 * C][:, sl]
        wo = Wo[sl, :]
        in_maps.append({
            "xt": _bf16(x[b].T),
            "wq": _bf16(wq.reshape(NCC, 128, FL)),
            "wk": _bf16(wk.reshape(NCC, 128, FL)),
            "wv": _bf16(wv.reshape(NCC, 128, FL)),
            "wo": _bf16(wo.reshape(NHP, 128, C)),
            "bq": np.ascontiguousarray(bqkv[0 * C:1 * C][sl].reshape(NHP, 128).T),
            "bk": np.ascontiguousarray(bqkv[1 * C:2 * C][sl].reshape(NHP, 128).T),
            "ident": _bf16(ident),
            "negtri": _bf16(negtri),
            "identr": ident,
            "vones": _bf16(np.ones((128, NTB * NHL), dtype=np.float32)),
        })
    return in_maps


def _assemble(res, bias):
    out = np.empty((B, T, C), dtype=np.float32)
    for b in range(B):
        out[b, : T // 2] = res.results[2 * b]["zh"]
        out[b, T // 2:] = res.results[2 * b + 1]["zh"]
    out += bias[None, None, :]
    return out


def kernel(x, Wqkv, bqkv, Wo, bo):
    in_maps = _in_maps(x, Wqkv, bqkv, Wo, bo)
    res = run_bass_kernel_spmd(_get_nc(), in_maps, core_ids=list(range(8)))
    # v-bias and output bias fold to a constant vector: softmax weights sum to
    # 1, so y = P@(v+bv)/rs = P@v/rs + bv  ->  out += bo + bv @ Wo
    bqkv_ = np.asarray(bqkv, dtype=np.float32)
    bias = (np.asarray(bo, dtype=np.float32)
            + bqkv_[2 * C:3 * C] @ np.asarray(Wo, dtype=np.float32))
    return _assemble(res, bias)


def run_traced(x, Wqkv, bqkv, Wo, bo, trace_cores=None):
    in_maps = _in_maps(x, Wqkv, bqkv, Wo, bo)
    res = run_bass_kernel_spmd(
        _get_nc(), in_maps, core_ids=list(range(8)), trace=True,
        trace_cores=trace_cores,
    )
    return res


# revision 5
# speedup vs baseline: 1.0100x; 1.0024x over previous
"""Causal multi-head attention block (QKV proj -> causal MHA -> out proj) on 8 Trainium2
cores — v3.

Sharding: core = b*2 + hh handles batch b (of 4) and head-half hh (8 of 16 heads).
Host pre-transposes x (x^T) and pre-formats all weights as bf16, so the device does
only GEMMs + attention. All matmuls run in bf16 (fp32 PSUM accumulation); softmax is
fp32 on the Activation engine.

Structure per core:
  - Q^T/K^T [feat(part), tok] and V [tok(part), feat] projections from resident x^T.
  - scores^T tiles [k(part), q] with the causal mask added via an extra accumulate
    matmul (identity x neg-triangle) on the PE, then exp on Act into bf16 P tiles.
  - AV with P stationary: Y[q(part), 65] accumulated over k-blocks; col 64 is the
    softmax denominator via a ones-column in V. Normalize = DVE reciprocal +
    per-partition tensor_scalar multiply; y^T via PE transposes; out-proj into
    token-major z with a pairwise ReduceScatter.
  - The Act engine runs only exp and is the secondary bottleneck; attention units
    are interleaved with fine-grained QKV / out-proj filler via a deficit model so
    the PE never waits on exp.
"""

import numpy as np

import concourse.bass as bass
import concourse.tile as tile
from concourse import bacc, mybir
from concourse.bass_utils import run_bass_kernel_spmd

F32 = mybir.dt.float32
F32R = mybir.dt.float32r
BF16 = mybir.dt.bfloat16
AF = mybir.ActivationFunctionType

B, T, C, H = 4, 2048, 1024, 16
D = C // H          # 64
NHL = H // 2        # 8 local heads per core
NHP = NHL // 2      # 4 local head pairs
FL = NHL * D        # 512 local features
NCC = C // 128      # 8 contraction chunks over C
NTB = T // 128      # 16 token blocks
NTT = T // 512      # 4 token tiles / qtiles
VE = D + 1          # 65: V feature cols + ones column
NEG = -1.0e30

# zpart row remap so each pairwise-RS chunk is a contiguous 256-row block:
# chunk c holds tb c (rank0 tokens) then tb 8+c (rank1 tokens)
ZROW = {}
for _c in range(8):
    ZROW[_c] = _c * 256
    ZROW[8 + _c] = _c * 256 + 128


def build():
    nc = bacc.Bacc("TRN2", target_bir_lowering=False, num_devices=8)

    xt_d = nc.dram_tensor("xt", [C, T], BF16, kind="ExternalInput")
    wq_d = nc.dram_tensor("wq", [NCC, 128, FL], BF16, kind="ExternalInput")
    wk_d = nc.dram_tensor("wk", [NCC, 128, FL], BF16, kind="ExternalInput")
    wv_d = nc.dram_tensor("wv", [NCC, 128, FL], BF16, kind="ExternalInput")
    wo_d = nc.dram_tensor("wo", [NHP, 128, C], BF16, kind="ExternalInput")
    bq_d = nc.dram_tensor("bq", [128, NHP], F32, kind="ExternalInput")
    bk_d = nc.dram_tensor("bk", [128, NHP], F32, kind="ExternalInput")
    id_d = nc.dram_tensor("ident", [128, 128], BF16, kind="ExternalInput")
    neg_d = nc.dram_tensor("negtri", [128, 128], BF16, kind="ExternalInput")
    idr_d = nc.dram_tensor("identr", [128, 128], F32R, kind="ExternalInput")
    ones_d = nc.dram_tensor("vones", [128, NTB * NHL], BF16, kind="ExternalInput")
    zh = nc.dram_tensor("zh", [T // 2, C], F32, kind="ExternalOutput")

    with tile.TileContext(nc) as tc:
        with (
            tc.tile_pool(name="res", bufs=1) as res,
            tc.tile_pool(name="cst", bufs=1) as cst,
            tc.tile_pool(name="dram", bufs=1, space="DRAM") as dram,
            tc.tile_pool(name="mp", bufs=2, space="PSUM") as mp,       # 2 banks
            tc.tile_pool(name="sp", bufs=2, space="PSUM") as sp,       # 4 banks
            tc.tile_pool(name="yp", bufs=2, space="PSUM") as yp,       # 2 banks
            tc.tile_pool(name="pp", bufs=8) as pp,
            tc.tile_pool(name="yb", bufs=6) as yb,
            tc.tile_pool(name="rc", bufs=6) as rc,
            tc.tile_pool(name="zb", bufs=3) as zb,
        ):
            # ---------------- resident tiles ----------------
            xt = res.tile([128, NCC * T], BF16)          # x^T  [c | cc, tok]
            xt3 = xt[:].rearrange("p (c t) -> p c t", c=NCC)
            qt_sb = res.tile([128, NHP * T], BF16)       # Q^T  [d(2 heads) | hp, tok]
            qt3 = qt_sb[:].rearrange("p (h t) -> p h t", h=NHP)
            kt_sb = res.tile([128, NHP * T], BF16)       # K^T
            kt3 = kt_sb[:].rearrange("p (h t) -> p h t", h=NHP)
            v_sb = res.tile([128, NTB * NHL * VE], BF16)  # V' [k | tb, h, d+1]
            v4 = v_sb[:].rearrange("p (b h e) -> p b h e", b=NTB, h=NHL)
            yt_sb = res.tile([128, NHP * T], BF16)       # y^T [f(2 heads) | hp, tok]
            yt3 = yt_sb[:].rearrange("p (h t) -> p h t", h=NHP)

            zpart = dram.tile([T, C], F32, name="zpart")
            zreds = [dram.tile([128, C], F32, name=f"zred{i}") for i in range(8)]

            # ------------- loads, ordered so K(tt0)/V(tt0) unblock fastest -------------
            scr = cst.tile([128, 128], BF16, tag="scr")
            nc.vector.memset(scr[:], 0.0)
            nc.sync.dma_start(
                xt3[:, :, 0:512],
                xt_d[:, 0:512].rearrange("(c p) t -> p c t", p=128),
            )
            wk_sb = cst.tile([128, NCC * FL], BF16, tag="wk")
            wk3 = wk_sb[:].rearrange("p (c f) -> p c f", c=NCC)
            nc.sync.dma_start(
                wk3[:, :, 0:256], wk_d[:, :, 0:256].rearrange("c p f -> p c f")
            )
            bk_sb = cst.tile([128, NHP], F32, tag="bk")
            nc.sync.dma_start(bk_sb[:], bk_d[:, :])
            nc.sync.dma_start(
                wk3[:, :, 256:512], wk_d[:, :, 256:512].rearrange("c p f -> p c f")
            )
            id_sb = cst.tile([128, 128], BF16, tag="id")
            nc.sync.dma_start(id_sb[:], id_d[:, :])
            neg_sb = cst.tile([128, 128], BF16, tag="neg")
            nc.sync.dma_start(neg_sb[:], neg_d[:, :])
            idr_sb = cst.tile([128, 128], F32R, tag="idr")
            nc.sync.dma_start(idr_sb[:], idr_d[:, :])
            bq_sb = cst.tile([128, NHP], F32, tag="bq")
            nc.sync.dma_start(bq_sb[:], bq_d[:, :])
            wv_sb = cst.tile([128, NCC * FL], BF16, tag="wv")
            wv3 = wv_sb[:].rearrange("p (c f) -> p c f", c=NCC)
            nc.sync.dma_start(wv3, wv_d[:, :, :].rearrange("c p f -> p c f"))
            wq_sb = cst.tile([128, NCC * FL], BF16, tag="wq")
            wq3 = wq_sb[:].rearrange("p (c f) -> p c f", c=NCC)
            nc.sync.dma_start(wq3, wq_d[:, :, :].rearrange("c p f -> p c f"))
            # warm the exp table set (hides ACT_TABLE_LOAD)
            warm = cst.tile([1, 1], F32, tag="warm")
            nc.scalar.activation(warm[:], bq_sb[0:1, 0:1], AF.Exp)
            # ones column of V' via a small DMA + one strided DVE copy
            ones_sb = cst.tile([128, NTB * NHL], BF16, tag="ones")
            nc.sync.dma_start(ones_sb[:], ones_d[:, :])
            nc.vector.tensor_copy(
                v4[:, :, :, D:D + 1],
                ones_sb[:].rearrange("p (b h) -> p b h", b=NTB).unsqueeze(3),
            )
            # PE p-state warm-up while the x^T/weight DMAs land (scratch is
            # memset-initialized, so no DMA gates the first matmul)
            wps = sp.tile([128, 1024], F32, tag="s", name="warmps")
            for wi in range(80):
                nc.tensor.matmul(
                    wps[:, (wi % 8) * 128:(wi % 8) * 128 + 128],
                    scr[:], scr[:], start=True, stop=True,
                )
            for tt in range(1, NTT):
                nc.sync.dma_start(
                    xt3[:, :, tt * 512:(tt + 1) * 512],
                    xt_d[:, tt * 512:(tt + 1) * 512].rearrange(
                        "(c p) t -> p c t", p=128
                    ),
                )
            wo_sb = cst.tile([128, NHP * C], BF16, tag="wo")
            wo3 = wo_sb[:].rearrange("p (h f) -> p h f", h=NHP)
            nc.sync.dma_start(wo3, wo_d[:, :, :].rearrange("h p f -> p h f"))

            # ---------------- fine-grained emission units ----------------
            # Each filler unit is (pe_ns_estimate, closure). The scheduler pops
            # fillers to cover the Act-vs-PE deficit inside attention units.
            def u_qk(tt, fb, w3, b_sb, dst3, nm):
                key = (nm, tt, fb)
                def go():
                    ps = mp.tile([128, 512], F32, tag="mp", name=f"{nm}{tt}_{fb}")
                    for cc in range(NCC):
                        nc.tensor.matmul(
                            ps[:],
                            w3[:, cc, fb * 128:(fb + 1) * 128],
                            xt3[:, cc, tt * 512:(tt + 1) * 512],
                            start=(cc == 0),
                            stop=(cc == NCC - 1),
                        )
                    nc.vector.tensor_scalar_add(
                        dst3[:, fb, tt * 512:(tt + 1) * 512], ps[:],
                        b_sb[:, fb:fb + 1],
                    )
                return (key, 1707, go)

            def u_v(tb):
                key = ("vt", tb)
                def go():
                    ps = mp.tile([128, 512], F32, tag="mp", name=f"v{tb}")
                    for cc in range(NCC):
                        nc.tensor.matmul(
                            ps[:],
                            xt3[:, cc, tb * 128:(tb + 1) * 128],
                            wv3[:, cc, :],
                            start=(cc == 0),
                            stop=(cc == NCC - 1),
                        )
                    nc.vector.tensor_copy(
                        v4[:, tb, :, 0:D],
                        ps[:].rearrange("p (h d) -> p h d", h=NHL),
                    )
                return (key, 1707, go)

            def u_op(tb, ct, on_act=False):
                key = ("op", tb, ct)
                def go():
                    zps = mp.tile([128, 512], F32, tag="mp", name=f"z{tb}_{ct}")
                    for cc in range(NHP):
                        nc.tensor.matmul(
                            zps[:],
                            yt3[:, cc, tb * 128:(tb + 1) * 128],
                            wo3[:, cc, ct * 512:(ct + 1) * 512],
                            start=(cc == 0),
                            stop=(cc == NHP - 1),
                        )
                    z_sb = zb.tile([128, 512], F32, tag="z", name=f"zs{tb}_{ct}")
                    if on_act:
                        nc.scalar.copy(z_sb[:], zps[:])
                    else:
                        nc.vector.tensor_copy(z_sb[:], zps[:])
                    nc.sync.dma_start(
                        zpart[ZROW[tb]:ZROW[tb] + 128, ct * 512:(ct + 1) * 512],
                        z_sb[:],
                    )
                return (key, 880, go)

            fillers = []          # deque of (key, pe_ns, closure)
            done_keys = set()
            deficit = [0.0]       # act-over-pe backlog inside attention

            def fill(need_ns):
                deficit[0] += need_ns
                while fillers and deficit[0] > fillers[0][1] * 1.5:
                    key, pe_ns, go = fillers.pop(0)
                    go()
                    done_keys.add(key)
                    deficit[0] -= pe_ns

            def require(*keys):
                # run exactly the queued fillers the caller depends on
                need = {k for k in keys if k not in done_keys}
                if not need:
                    return
                rest = []
                forced = []
                for key, pe_ns, go in fillers:
                    if key in need:
                        go()
                        done_keys.add(key)
                        deficit[0] -= pe_ns
                        need.discard(key)
                        forced.append(key)
                    else:
                        rest.append((key, pe_ns, go))
                fillers[:] = rest

                if need:
                    raise RuntimeError(f"missing filler deps: {need}")

            pend_tp = []          # pending y-transpose closures (pair-complete)

            def flush_tp(keep=0):
                while len(pend_tp) > keep:
                    pend_tp.pop(0)()

            def emit_att_pair(he, qt):
                # both heads of a pair interleaved at kb-pair granularity:
                # head B's scores cover head A's exp latency and vice versa
                hp = he // 2
                n_kb = 4 * (qt + 1)
                require(("q", qt, hp),
                        *[("k", t, hp) for t in range(qt + 1)],
                        *[("vt", tb) for tb in range(4 * qt + 4)])
                flush_tp(keep=1)
                Ys = [yp.tile([128, 512], F32, tag="y", name=f"Y{he + i}_{qt}")
                      for i in range(2)]
                Y3s = [Y[:, 0:4 * VE].rearrange("p (q e) -> p q e", q=4)
                       for Y in Ys]
                for kbp in range(n_kb // 2):
                    Ps = []
                    css = None
                    act_ns = 0.0
                    pe_ns = 0.0
                    for hi in range(2):
                        s = sp.tile([128, 1024], F32, tag="s",
                                    name=f"s{he + hi}_{qt}_{kbp}")
                        cs = []
                        for c2 in range(2):
                            kb = 2 * kbp + c2
                            c = kb - 4 * qt
                            cs.append(c)
                            j0 = c * 128 if c > 0 else 0
                            nc.tensor.matmul(
                                s[:, c2 * 512 + j0:(c2 + 1) * 512],
                                kt3[hi * 64:(hi + 1) * 64, hp,
                                    kb * 128:(kb + 1) * 128],
                                qt3[hi * 64:(hi + 1) * 64, hp,
                                    qt * 512 + j0:(qt + 1) * 512],
                                tile_position=(hi * 64, 0),
                                start=True,
                                stop=not (0 <= c <= 3),
                            )
                            pe_ns += (512 - j0) * 0.4167
                            if 0 <= c <= 3:
                                nc.tensor.matmul(
                                    s[:, c2 * 512 + c * 128:
                                      c2 * 512 + (c + 1) * 128],
                                    id_sb[:],
                                    neg_sb[:],
                                    start=False,
                                    stop=True,
                                )
                                pe_ns += 128 * 0.4167
                        css = cs
                        P = pp.tile([128, 1024], BF16, tag="p",
                                    name=f"p{he + hi}_{qt}_{kbp}")
                        if 0 <= cs[0] <= 3:
                            ja = cs[0] * 128
                            jb = 512 + cs[1] * 128
                            nc.scalar.activation(
                                P[:, ja:512], s[:, ja:512], AF.Exp, scale=0.125
                            )
                            nc.scalar.activation(
                                P[:, jb:], s[:, jb:], AF.Exp, scale=0.125
                            )
                            act_ns += (1536 - ja - jb) * 0.833 + 300
                        else:
                            nc.scalar.activation(P[:], s[:], AF.Exp, scale=0.125)
                            act_ns += 1024 * 0.833 + 150
                        Ps.append(P)
                    n_av = sum(1 for c2_ in range(2) for qb_ in range(4)
                               if (2 * kbp + c2_ - 4 * qt) <= qb_)
                    fill(max(0.0, act_ns - pe_ns - 2 * n_av * VE * 0.4167))
                    for hi in range(2):
                        for c2 in range(2):
                            kb = 2 * kbp + c2
                            c = kb - 4 * qt
                            for qb in range(4):
                                if c > qb:
                                    continue
                                nc.tensor.matmul(
                                    Y3s[hi][:, qb, :],
                                    Ps[hi][:, c2 * 512 + qb * 128:
                                           c2 * 512 + (qb + 1) * 128],
                                    v4[:, kb, he + hi, :],
                                    start=(kb == 0 and qb == 0),
                                    stop=(kb == n_kb - 1 and qb == 3),
                                )
                # normalize both heads, then defer the pair transpose
                ys = []
                for hi in range(2):
                    r = rc.tile([128, 4], F32, tag="r", name=f"r{he + hi}_{qt}")
                    nc.vector.reciprocal(
                        r[:], Y3s[hi][:, :, D:D + 1].rearrange("p q e -> p (q e)")
                    )
                    y = yb.tile([128, 4 * D], BF16, tag="yb",
                                name=f"y{he + hi}_{qt}")
                    y3 = y[:].rearrange("p (q d) -> p q d", q=4)
                    for qb in range(4):
                        nc.vector.tensor_scalar(
                            y3[:, qb, :], Y3s[hi][:, qb, 0:D], r[:, qb:qb + 1],
                            None, mybir.AluOpType.mult,
                        )
                    ys.append(y)

                def tp_go(ys=ys, hp=hp, qt=qt):
                    tp = sp.tile([128, 2048], BF16, tag="s", name=f"yt{hp}_{qt}")
                    for hi in range(2):
                        y3 = ys[hi][:].rearrange("p (q d) -> p q d", q=4)
                        for qb in range(4):
                            nc.tensor.transpose(
                                tp[hi * 64:(hi + 1) * 64,
                                   qb * 128:(qb + 1) * 128],
                                y3[:, qb, :], id_sb[:],
                            )
                    nc.vector.tensor_copy(
                        yt3[:, hp, qt * 512:(qt + 1) * 512], tp[:, 0:512]
                    )
                pend_tp.append(tp_go)

            def emit_rs(c):
                nc.gpsimd.collective_compute(
                    "ReduceScatter",
                    mybir.AluOpType.add,
                    replica_groups=[[0, 1], [2, 3], [4, 5], [6, 7]],
                    ins=[zpart[c * 256:(c + 1) * 256, :].opt()],
                    outs=[zreds[c].opt()],
                )
                nc.sync.dma_start(zh[c * 128:(c + 1) * 128, :], zreds[c][:])

            # ---------------- static schedule ----------------
            def push(*units):
                fillers.extend(units)

            def run_now(*units):
                for key, _, go in units:
                    go()
                    done_keys.add(key)

            # K/V/Q for tt0 emitted directly (attention qt0 needs them)
            run_now(*[u_qk(0, fb, wk3, bk_sb, kt3, "k") for fb in range(NHP)])
            run_now(*[u_v(tb) for tb in range(4)])
            run_now(*[u_qk(0, fb, wq3, bq_sb, qt3, "q") for fb in range(NHP)])

            push(*[u_qk(1, fb, wk3, bk_sb, kt3, "k") for fb in range(NHP)])
            push(*[u_v(tb) for tb in range(4, 8)])
            push(*[u_qk(1, fb, wq3, bq_sb, qt3, "q") for fb in range(NHP)])
            push(*[u_qk(2, fb, wk3, bk_sb, kt3, "k") for fb in range(NHP)])
            push(*[u_v(tb) for tb in range(8, 12)])
            push(*[u_qk(2, fb, wq3, bq_sb, qt3, "q") for fb in range(NHP)])

            for he in range(0, NHL, 2):
                emit_att_pair(he, 0)
            push(*[u_qk(3, fb, wk3, bk_sb, kt3, "k") for fb in range(NHP)])
            push(*[u_v(tb) for tb in range(12, 16)])
            push(*[u_qk(3, fb, wq3, bq_sb, qt3, "q") for fb in range(NHP)])
            flush_tp()

            for he in range(0, NHL, 2):
                emit_att_pair(he, 1)
            flush_tp()
            push(*[u_op(tb, ct) for tb in range(0, 8) for ct in range(2)])

            order = [(0, 2), (0, 3), (2, 3), (2, 2), (4, 3), (4, 2),
                     (6, 3), (6, 2)]
            for he, qt in order:
                emit_att_pair(he, qt)
            flush_tp()
            push(*[u_op(tb, ct, on_act=(ct == 1)) for tb in range(8, 12)
                   for ct in range(2)])

            # drain remaining fillers (anything the deficit model didn't pull)
            while fillers:
                fillers.pop(0)[2]()
            for c in range(4):
                emit_rs(c)
            for tb in range(12, 16):
                # drain-region out-proj: both halves in one wide s-pool psum
                # tile, copies split across DVE and Act so they overlap
                zps = sp.tile([128, 1024], F32, tag="s", name=f"zw{tb}")
                for ct in range(2):
                    for cc in range(NHP):
                        nc.tensor.matmul(
                            zps[:, ct * 512:(ct + 1) * 512],
                            yt3[:, cc, tb * 128:(tb + 1) * 128],
                            wo3[:, cc, ct * 512:(ct + 1) * 512],
                            start=(cc == 0),
                            stop=(cc == NHP - 1),
                        )
                z_sb = zb.tile([128, 1024], F32, tag="zw", name=f"zsw{tb}")
                nc.vector.tensor_copy(z_sb[:, 0:512], zps[:, 0:512])
                nc.sync.dma_start(
                    zpart[ZROW[tb]:ZROW[tb] + 128, 0:512], z_sb[:, 0:512]
                )
                nc.scalar.copy(z_sb[:, 512:1024], zps[:, 512:1024])
                nc.sync.dma_start(
                    zpart[ZROW[tb]:ZROW[tb] + 128, 512:1024], z_sb[:, 512:1024]
                )
                emit_rs(tb - 8)

    nc.compile()
    return nc


_NC_CACHE = None


def _get_nc():
    global _NC_CACHE
    if _NC_CACHE is None:
        _NC_CACHE = build()
    return _NC_CACHE


def _bf16(a):
    import ml_dtypes
    return np.ascontiguousarray(a).astype(ml_dtypes.bfloat16)


def _in_maps(x, Wqkv, bqkv, Wo, bo):
    x = np.asarray(x, dtype=np.float32)
    Wqkv = np.asarray(Wqkv, dtype=np.float32)
    bqkv = np.asarray(bqkv, dtype=np.float32)
    Wo = np.asarray(Wo, dtype=np.float32)
    bo = np.asarray(bo, dtype=np.float32)

    ident = np.eye(128, dtype=np.float32)
    i_ = np.arange(128)[:, None]
    j_ = np.arange(128)[None, :]
    negtri = np.where(i_ > j_, np.float32(NEG), np.float32(0.0))

    in_maps = []
    for core in range(8):
        b, hh = core // 2, core % 2
        sl = slice(hh * FL, (hh + 1) * FL)
        wq = Wqkv[:, 0 * C:1 * C][:, sl]
        wk = Wqkv[:, 1 * C:2 * C][:, sl]
        wv = Wqkv[:, 2 * C:3 * C][:, sl]
        wo = Wo[sl, :]
        in_maps.append({
            "xt": _bf16(x[b].T),
            "wq": _bf16(wq.reshape(NCC, 128, FL)),
            "wk": _bf16(wk.reshape(NCC, 128, FL)),
            "wv": _bf16(wv.reshape(NCC, 128, FL)),
            "wo": _bf16(wo.reshape(NHP, 128, C)),
            "bq": np.ascontiguousarray(bqkv[0 * C:1 * C][sl].reshape(NHP, 128).T),
            "bk": np.ascontiguousarray(bqkv[1 * C:2 * C][sl].reshape(NHP, 128).T),
            "ident": _bf16(ident),
            "negtri": _bf16(negtri),
            "identr": ident,
            "vones": _bf16(np.ones((128, NTB * NHL), dtype=np.float32)),
        })
    return in_maps


def _assemble(res, bias):
    out = np.empty((B, T, C), dtype=np.float32)
    for b in range(B):
        out[b, : T // 2] = res.results[2 * b]["zh"]
        out[b, T // 2:] = res.results[2 * b + 1]["zh"]
    out += bias[None, None, :]
    return out


def kernel(x, Wqkv, bqkv, Wo, bo):
    in_maps = _in_maps(x, Wqkv, bqkv, Wo, bo)
    res = run_bass_kernel_spmd(_get_nc(), in_maps, core_ids=list(range(8)))
    # v-bias and output bias fold to a constant vector: softmax weights sum to
    # 1, so y = P@(v+bv)/rs = P@v/rs + bv  ->  out += bo + bv @ Wo
    bqkv_ = np.asarray(bqkv, dtype=np.float32)
    bias = (np.asarray(bo, dtype=np.float32)
            + bqkv_[2 * C:3 * C] @ np.asarray(Wo, dtype=np.float32))
    return _assemble(res, bias)


def run_traced(x, Wqkv, bqkv, Wo, bo, trace_cores=None):
    in_maps = _in_maps(x, Wqkv, bqkv, Wo, bo)
    res = run_bass_kernel_spmd(
        _get_nc(), in_maps, core_ids=list(range(8)), trace=True,
        trace_cores=trace_cores,
    )
    return res
